# revision 59
# baseline (speedup 1.0000x reference)
"""Trainium2 Bass kernel for nn_LogReg (LayerNorm -> Linear(256,128)+Sigmoid -> Linear(128,10)).

Data-parallel over 8 NeuronCores: the 1408-row batch is split into 8 shards of
176 rows; the small LN/Linear parameters are replicated to every core.

Layout strategy (host side does pure relayout only -- slicing, reshape,
transpose, concatenation -- no arithmetic):
  * the seq shard ships PRE-TRANSPOSED as xT [128, 352]: partition d holds
    x[:, d] in cols 0:176 and x[:, 128+d] in cols 176:352. No on-chip
    transposes (and no identity matrix) are needed.
  * all parameters ship in one packed [128, 408] tensor: fc_w.T chunks,
    mlp_w.T, ln_g / ln_b column chunks, and fc_b / mlp_b as row vectors.
  * the output leaves as one [88, 20] tile (cols 0:10 = rows 0:88,
    cols 10:20 = rows 88:176) which the host unstacks to [176, 10].

Math: LayerNorm is folded into the FC matmul algebra so the big GEMM runs on
raw (un-normalized) data and the per-row corrections apply afterwards:

  pre[f,r] = ((Wg @ xT)[f,r] - mu[r]*w1[f]) / srt[r]
  hT       = sigmoid(pre + d)          (d applied as the ACT per-partition bias)

with Wg = fc_w * ln_g, w1 = rowsum(Wg), mu = s/256, srt = sqrt(var+eps),
d = fc_w @ ln_b + fc_b. Per-row stats come from bf16 PE matmuls against a
ones column (s = 1.T @ xT, q = 1.T @ xT^2); a rank-1 matmul accumulates
-s^2/256 onto the q accumulator so its PSUM tile closes as 256*var; the
mean correction is a rank-1 matmul (lhsT = -w1/256 row, rhs = s row); the
rstd division broadcasts srt across partitions with a ones outer-product
matmul into the G tile's second half, then one DVE reciprocal + one DVE
multiply produce pre. Matmuls run in bf16 (1 PE cycle/row vs fp32's 4);
stats scalars stay f32 in PSUM. d is computed from the raw f32 params.

Scheduling constraints honored throughout (each cost a failed compile to
learn): walrus allows a single semaphore-wait slot per instruction --
same-engine data deps also emit waits, matmuls get two slots (Ldweights +
Matmult) which absorb the joins; DVE ops may read at most one PSUM operand
(the G tile and srt broadcast share one PSUM tile so the readout's wait
covers both); GPSIMD cannot touch PSUM; DVE has no divide ALU; a start=True
matmul must never interleave into another group on the same PSUM bank; the
kernel-tail drain is re-emitted as single-wait SP no-ops with DMA-queue
waits skipped -- the drain itself quiesces the DMA queues
(_SplitDrainTileContext).
"""

import numpy as np

import concourse.bass as bass
import concourse.mybir as mybir
import concourse.tile as tile
from concourse.bass_utils import run_bass_kernel_spmd
from concourse.vector_clock import ScopedClock


class _SplitDrainTileContext(tile.TileContext):
    """TileContext whose kernel-tail drain carries no semaphore waits.

    The walrus build in this environment supports a single wait slot per
    instruction, but the stock tail drain aggregates one wait per live
    semaphore. Re-emit those waits as individual single-wait instructions on
    the SP queue (in-order), then issue a bare drain.
    """

    # When True, skip the explicit waits on HWDGE queue semaphores before the
    # tail drain: the Drain instruction itself quiesces the DMA queues, and
    # the semaphore-propagation delay (~900 ns) would serialize on top.
    skip_dma_waits = True

    def _drain_and_barrier(self, tick_clock, wait_clock):
        nc = self.nc
        probe = mybir.InstNoOp(name=f"drain-probe-{nc.next_id()}", ins=[], outs=[])
        probe.engine = mybir.EngineType.SP
        wait_clock.add_sem_waits(probe, ScopedClock({None: tick_clock.global_clock}))
        pairs = []
        if probe.sync_info is not None:
            for w in probe.sync_info.on_wait or []:
                pairs.append((w.ant_name, w.wait_value))
        assert self.sems is not None
        by_name = {h.name: h for h in self.sems.allocated().values()}
        for name, val in pairs:
            if self.skip_dma_waits and name.startswith("DMAHW"):
                continue
            nc.sync.wait_ge(by_name[name], val)
        nc.sync.drain()
        nc.all_engine_barrier()
        popped = nc._tile_sem_poison_stack.pop()
        assert popped is self._sem_poison
        nc.clear_and_free_semaphores(list(self.sems.allocated().values()))
        nc.all_engine_barrier()


N_CORES = 8
ROWS = 1408
R = ROWS // N_CORES  # 176 rows per core
D = 256              # input feature dim
H = 128              # fc hidden dim
C = 10               # classes
P = 128              # SBUF partitions
KD = D // P          # contraction chunks for the 256-dim matmul
RH = R // 2          # 88-row halves for the final 128->10 matmul
LN_EPS = 1e-5
F32 = mybir.dt.float32
BF16 = mybir.dt.bfloat16

# packed params column layout
PFW = 0               # fc_w.T chunks        [128, 256]
PMW = PFW + D         # mlp_w.T              [128, 10]
PG = PMW + C          # ln_g chunk columns   [128, 2]
PB = PG + KD          # ln_b chunk columns   [128, 2]
PFCB = PB + KD        # fc_b row (row 0)     [1, 128]
PMB = PFCB + H        # mlp_b row (row 0)    [1, 10]
NC_PARAMS = PMB + C   # 408

TRACE = False
LAST_RESULTS = None

_cached_nc = None


def _build_nc() -> bass.Bass:
    nc = bass.Bass(trn_type="TRN2")

    seq = nc.dram_tensor("seq", [P, KD * R], F32, kind="ExternalInput")[:]
    params = nc.dram_tensor("params", [P, NC_PARAMS], F32, kind="ExternalInput")[:]
    out = nc.dram_tensor("out", [RH, 2 * C], F32, kind="ExternalOutput")[:]

    ACT = mybir.ActivationFunctionType
    ALU = mybir.AluOpType

    with _SplitDrainTileContext(nc) as tc:
        with (
            tc.tile_pool(name="singles", bufs=1) as singles,
            tc.tile_pool(name="psS", bufs=1, space="PSUM") as psS,
            tc.tile_pool(name="psQ", bufs=1, space="PSUM") as psQ,
            tc.tile_pool(name="psW", bufs=1, space="PSUM") as psW,
            tc.tile_pool(name="psD", bufs=1, space="PSUM") as psD,
            tc.tile_pool(name="psG", bufs=1, space="PSUM") as psG,
            tc.tile_pool(name="psP", bufs=1, space="PSUM") as psP,
            tc.tile_pool(name="psO", bufs=1, space="PSUM") as psO,
        ):
            # ---- input DMAs (SP queue): xT first (it heads the critical
            # path), params second ----
            xt = singles.tile([P, KD * R], F32, tag="xt")
            nc.sync.dma_start(out=xt[:], in_=seq)
            pp = singles.tile([P, NC_PARAMS], F32, tag="pp")
            nc.sync.dma_start(out=pp[:], in_=params)

            # ---- small constants (DVE memsets, before any data-dependent DVE
            # op so later engine waits subsume them) ----
            onesb_col = singles.tile([P, 1], BF16, tag="onesb_col")
            nc.vector.memset(onesb_col[:], 1.0)
            neginv_col = singles.tile([P, 1], BF16, tag="neginv_col")
            nc.vector.memset(neginv_col[:], -1.0 / D)
            onesb_row = singles.tile([1, R], BF16, tag="onesb_row")
            nc.vector.memset(onesb_row[:], 1.0)
            negone11 = singles.tile([1, 1], BF16, tag="negone11")
            nc.vector.memset(negone11[:], -1.0)
            ones_f32 = singles.tile([1, P], F32, tag="ones_f32")
            nc.vector.memset(ones_f32[:], 1.0)
            eps = singles.tile([1, 1], F32, tag="eps")
            nc.vector.memset(eps[:], LN_EPS)
            # ACT covering op: waits on the DVE memsets early so srt's eps
            # bias read is already subsumed when srt issues (single-wait rule)
            eps_scr = singles.tile([1, 1], F32, tag="eps_scr")
            nc.scalar.copy(out=eps_scr[:], in_=eps[:])

            fwT = [pp[:, PFW + k * P:PFW + (k + 1) * P] for k in range(KD)]
            mwT = pp[:, PMW:PMW + C]
            gT = [pp[:, PG + k:PG + k + 1] for k in range(KD)]
            bT = [pp[:, PB + k:PB + k + 1] for k in range(KD)]
            fcb_row = pp[0:1, PFCB:PFCB + H]
            mb_row = pp[0:1, PMB:PMB + C]

            # ---- bf16 stages of x: straight convert (split per chunk so the
            # first s matmul starts off chunk 0 early) + square, all on DVE
            # (ACT's slot goes to b2/srt; the square frees ACT entirely) ----
            xTb = singles.tile([P, KD * R], BF16, tag="xTb")
            for k in range(KD):
                nc.vector.tensor_copy(
                    out=xTb[:, k * R:(k + 1) * R], in_=xt[:, k * R:(k + 1) * R]
                )
            xsqb = singles.tile([P, KD * R], BF16, tag="xsqb")
            with nc.allow_low_precision(
                reason="bf16 x^2 feeds the bf16 sq-sum matmuls; tol 2e-2"
            ):
                nc.vector.tensor_tensor(
                    out=xsqb[:], in0=xt[:], in1=xt[:], op=ALU.mult
                )

            # ---- gain-folded FC weights, bf16 (both on DVE: ACT is owned by
            # the xsq/b2/srt chain in this window) ----
            wgTb = [
                singles.tile([P, P], BF16, tag=f"wgTb{k}", name=f"wgTb{k}")
                for k in range(KD)
            ]
            for k in range(KD):
                nc.vector.tensor_scalar_mul(
                    out=wgTb[k][:], in0=fwT[k], scalar1=gT[k]
                )

            # ---- PE: row sums s = 1.T @ xT and sq-sums q = 1.T @ xT^2 ----
            ps_s = psS.tile([1, R], F32, tag="ps_s")
            for k in range(KD):
                nc.tensor.matmul(
                    ps_s[:], lhsT=onesb_col[:], rhs=xTb[:, k * R:(k + 1) * R],
                    start=(k == 0), stop=(k == KD - 1),
                )
            # ps_q accumulates q = sum(x^2); a later rank-1 matmul folds in
            # -s^2/256, closing the group with ps_q = 256*var.
            ps_q = psQ.tile([1, R], F32, tag="ps_q")
            for k in range(KD):
                nc.tensor.matmul(
                    ps_q[:], lhsT=onesb_col[:], rhs=xsqb[:, k * R:(k + 1) * R],
                    start=(k == 0), stop=False,
                )

            # ---- PE: d = fc_w @ ln_b + fc_b as a [128, 1] column (raw f32
            # params; output free size 1 -> ~free on PE) ----
            ps_d = psD.tile([P, 1], F32, tag="ps_d")
            for k in range(KD):
                nc.tensor.matmul(
                    ps_d[:], lhsT=fwT[k], rhs=bT[k], start=(k == 0), stop=False
                )
            nc.tensor.matmul(
                ps_d[:], lhsT=fcb_row, rhs=ones_f32[:, 0:1], start=False, stop=True
            )

            # ---- PE: w1 = -rowsum(Wg)/256 as a [1, 128] row (the -1/256
            # lhsT pre-scales so the aug matmul can use the raw s row) ----
            ps_w1 = psW.tile([1, P], F32, tag="ps_w1")
            for k in range(KD):
                nc.tensor.matmul(
                    ps_w1[:], lhsT=neginv_col[:], rhs=wgTb[k][:],
                    start=(k == 0), stop=(k == KD - 1),
                )

            # ---- stats: b2 = (s/16)^2 on ACT, then a rank-1 matmul
            # accumulates -s^2/256 onto ps_q -> ps_q = 256*var (the PE's two
            # wait slots absorb this join); srt = sqrt(var+eps) on ACT ----
            b2 = singles.tile([1, R], BF16, tag="b2")
            nc.scalar.activation(
                out=b2[:], in_=ps_s[:], func=ACT.Square, scale=1.0 / 16.0
            )
            nc.tensor.matmul(
                ps_q[:], lhsT=negone11[:], rhs=b2[:], start=False, stop=True
            )
            srtb = singles.tile([1, R], BF16, tag="srtb")
            nc.scalar.activation(
                out=srtb[:], in_=ps_q[:], func=ACT.Sqrt,
                scale=1.0 / D, bias=eps[:],
            )
            # s and w1 readouts to SBUF bf16 (both DVE, s first: the aug
            # matmul's single DVE wait on w1_sb then also covers s_b)
            s_b = singles.tile([1, R], BF16, tag="s_b")
            nc.vector.tensor_copy(out=s_b[:], in_=ps_s[:])
            w1_sb = singles.tile([1, P], BF16, tag="w1_sb")
            nc.vector.tensor_copy(out=w1_sb[:], in_=ps_w1[:])
            # bf16 copies of the 128->10 weights and bias (DVE; emitted late
            # so the scheduler doesn't let them delay the wgTb converts)
            mwTb = singles.tile([P, C], BF16, tag="mwTb")
            nc.vector.tensor_copy(out=mwTb[:], in_=mwT)
            mbb = singles.tile([1, C], BF16, tag="mbb")
            nc.vector.tensor_copy(out=mbb[:], in_=mb_row)

            # ---- PE: one shared PSUM tile [P, 2R]: cols 0:R accumulate
            # G = Wg @ xT - w1 (x) mu (rank-1 aug with the prescaled w1 row);
            # cols R:2R take the srt broadcast (ones outer product). Sharing
            # the tile makes the downstream readout wait on whichever matmul
            # lands last, keeping the divide to a single wait. ----
            ps_GR = psG.tile([P, 2 * R], F32, tag="ps_GR")
            for k in range(KD):
                nc.tensor.matmul(
                    ps_GR[:, 0:R], lhsT=wgTb[k][:], rhs=xTb[:, k * R:(k + 1) * R],
                    start=(k == 0), stop=False,
                )
            nc.tensor.matmul(
                ps_GR[:, 0:R], lhsT=w1_sb[:], rhs=s_b[:], start=False, stop=True
            )
            nc.tensor.matmul(
                ps_GR[:, R:2 * R], lhsT=onesb_row[:, 0:P], rhs=srtb[:],
                start=True, stop=True,
            )

            # ---- d column readout (DVE); reciprocal of the srt broadcast
            # (PSUM -> SBUF bf16); pre = G * (1/srt) on DVE ----
            d_t = singles.tile([P, 1], F32, tag="d_t")
            nc.vector.tensor_copy(out=d_t[:], in_=ps_d[:])
            sr_rec = singles.tile([P, R], BF16, tag="sr_rec")
            with nc.allow_low_precision(
                reason="bf16 1/srt feeds one elementwise mult; tol 2e-2"
            ):
                nc.vector.reciprocal(out=sr_rec[:], in_=ps_GR[:, R:2 * R])
            ps_pre = psP.tile([P, R], F32, tag="ps_pre")
            nc.vector.tensor_tensor(
                out=ps_pre[:], in0=ps_GR[:, 0:R], in1=sr_rec[:], op=ALU.mult
            )
            # ---- ACT: hT = sigmoid(pre + d) (PSUM read is cheaper) ----
            hT = singles.tile([P, R], BF16, tag="hT")
            with nc.allow_low_precision(
                reason="bf16 sigmoid output feeds a bf16 matmul; tol 2e-2"
            ):
                nc.scalar.activation(
                    out=hT[:], in_=ps_pre[:], func=ACT.Sigmoid, bias=d_t[:]
                )

            # ---- PE: out halves = hT_g.T @ mlp_wT + 1 x mlp_b (raw f32) ----
            ps2 = psO.tile([RH, 2, C], F32, tag="ps2")
            for g in range(2):
                cols = slice(g * RH, (g + 1) * RH)
                nc.tensor.matmul(
                    ps2[:, g, :], lhsT=hT[:, cols], rhs=mwTb[:],
                    start=True, stop=False,
                )
                nc.tensor.matmul(
                    ps2[:, g, :], lhsT=onesb_row[:, 0:RH], rhs=mbb[:],
                    start=False, stop=True,
                )

            # ---- DVE readout + output DMA ----
            ot = singles.tile([RH, 2, C], F32, tag="ot")
            nc.vector.tensor_copy(out=ot[:], in_=ps2[:])
            nc.sync.dma_start(
                out=out.rearrange("p (g c) -> p g c", g=2), in_=ot[:]
            )

    return nc


def kernel(seq, ln_g, ln_b, fc_w, fc_b, mlp_w, mlp_b):
    global _cached_nc, LAST_RESULTS
    seq = np.asarray(seq, dtype=np.float32)
    ln_g = np.asarray(ln_g, dtype=np.float32)
    ln_b = np.asarray(ln_b, dtype=np.float32)
    fc_w = np.asarray(fc_w, dtype=np.float32)
    fc_b = np.asarray(fc_b, dtype=np.float32)
    mlp_w = np.asarray(mlp_w, dtype=np.float32)
    mlp_b = np.asarray(mlp_b, dtype=np.float32)

    # Pack parameters (pure relayout) into one [128, 408] tensor.
    pk = np.zeros((P, NC_PARAMS), dtype=np.float32)
    fwt = fc_w.T  # [256, 128]; chunk k as lhsT: tile[p, j] = fc_w[j, k*128+p]
    for k in range(KD):
        pk[:, PFW + k * P:PFW + (k + 1) * P] = fwt[k * P:(k + 1) * P, :]
    pk[:, PMW:PMW + C] = mlp_w.T
    for k in range(KD):
        pk[:, PG + k] = ln_g[k * P:(k + 1) * P]
        pk[:, PB + k] = ln_b[k * P:(k + 1) * P]
    pk[0, PFCB:PFCB + H] = fc_b
    pk[0, PMB:PMB + C] = mlp_b

    if _cached_nc is None:
        _cached_nc = _build_nc()
    nc = _cached_nc

    in_maps = []
    for c in range(N_CORES):
        shard_t = seq[c * R:(c + 1) * R].T  # [256, 176], pure relayout
        xt_host = np.ascontiguousarray(
            np.concatenate([shard_t[0:P], shard_t[P:D]], axis=1)
        )  # [128, 352]
        in_maps.append({"seq": xt_host, "params": pk})

    res = run_bass_kernel_spmd(
        nc, in_maps, core_ids=list(range(N_CORES)), trace=TRACE
    )
    LAST_RESULTS = res
    # out shard [88, 20]: cols 0:10 = rows 0:88, cols 10:20 = rows 88:176
    shards = []
    for c in range(N_CORES):
        o = res.results[c]["out"].reshape(RH, 2 * C)
        shards.append(np.concatenate([o[:, 0:C], o[:, C:2 * C]], axis=0))
    full = np.concatenate(shards, axis=0)
    return full.reshape(32, 4, 11, C).astype(np.float32)
